# revision 22
# baseline (speedup 1.0000x reference)
"""Distributed Bass kernel for A2GNN propagation (GCN-normalized message
passing) on 8 TRN2 NeuronCores.

Scheme (node/data parallel):
  - Nodes range-sharded across 8 cores (12500 each, padded to 12544 rows).
  - The propagation state is kept in *pre-scaled* form y = dinv * x (the GCN
    norm dinv[s]*dinv[d] is separable), so edge messages need no per-edge
    scaling at all:  x_new[d] = dinv[d] * (sum_{s->d} y[s] + y[d]);
    y_new = dinv^2 * (sum y[s] + y[d]).
  - Per hop: each core AllGathers the 3.2 MB y-shard into a replicated
    [100352, 64] Shared table, dma_gathers the source rows for its edges
    (grouped into 4 table chunks so int16 indices fit), and reduces them
    into its shard accumulator with per-128-dst-block one-hot matmuls on
    the PE (selection matrices built on DVE by is_equal against an iota).
    The accumulator lives in SBUF; its start value (the previous y-shard)
    provides the self-loop term. dma_scatter_add is NOT used (its CCE add
    loses colliding updates on HW).
  - x@W0 and the relu(h+b0)@Wc classifier run on the PE between hop groups.

Perf notes (measured on trn2):
  - The hop is bound by dma_gather descriptor generation: the ANT gather
    ucode runs on ONE Q7 core pair per call (cpu_id/2 == queue_num; tx/rx
    split) at ~6 ns/index + ~1 us fixed, serialized on the Pool engine, so
    each 1024-index call costs ~6 us and 104 calls/hop ~640 us. num_idxs >
    1024 hard-crashes the ucode; multi-packet and indirect_dma_start are
    not faster. SWDGE queues are round-robined (NQ=4) so one queue's DMA
    drain overlaps the next call's descriptor generation (~1.5x).
  - The shard is split into two 49-block halves with separate AllGathers
    (tableA/tableB, double-buffered by hop parity). Each half's AG is
    issued as soon as that half of the accumulator is final, overlapping
    the remaining sessions and the next hop's gathers, which hides the
    whole collective (~84 us/hop exposed otherwise).
"""

import os
import sys

sys.path.insert(0, "/opt/trn_rl_repo")

import numpy as np


# ---------------------------------------------------------------- config ----
class Cfg:
    N = 100000          # nodes
    IND = 128           # input dim
    HID = 64            # hidden dim (and padded classifier dim)
    NCLS = 40           # classes
    NC = 8              # cores
    SHARD = N // NC     # 12500 owned nodes per core
    NBLK = (SHARD + 127) // 128          # 98 row-blocks per shard
    SHARD_PAD = NBLK * 128               # 12544
    TROWS = SHARD_PAD * NC               # 100352 table rows
    HBLK = NBLK // 2                     # 49 blocks per shard half
    HROWS = HBLK * 128                   # 6272 rows per shard half
    NCHUNK = 4
    CHUNK = TROWS // NCHUNK              # 25088 (< 32768 so int16 indices fit)
    SLICE_COLS = int(os.environ.get("GNN_SLICE_COLS", "8"))
    SELG = int(os.environ.get("GNN_SELG", "16"))  # columns per DVE Sel build
    PADQ = 32                            # segment token-count quantum
    NTABLES = int(os.environ.get("GNN_NTABLES", "2"))
    NQ = int(os.environ.get("GNN_NQ", "4"))           # SWDGE queues (1..4)
    SKIP_MM = bool(int(os.environ.get("GNN_SKIP_MM", "0")))
    SKIP_GATHER = bool(int(os.environ.get("GNN_SKIP_GATHER", "0")))
    SKIP_SEL = bool(int(os.environ.get("GNN_SKIP_SEL", "0")))


# ---------------------------------------------------------- host packing ----
def _pack_streams(cfg, edge_index):
    """Per-core token streams: (chunk, dst-block)-segmented, column-padded.

    Returns (streams, seги_meta) where seg_meta = (seg_cols, chunk_cols, T).
    """
    src = np.asarray(edge_index[0], dtype=np.int64)
    dst = np.asarray(edge_index[1], dtype=np.int64)
    N = cfg.N

    deg = (np.bincount(dst, minlength=N) + 1).astype(np.float64)  # + self loop
    dinv64 = 1.0 / np.sqrt(deg)
    dinv = dinv64.astype(np.float32)
    d2 = (dinv64 * dinv64).astype(np.float32)
    dinvi = np.sqrt(deg).astype(np.float32)

    # per-core, per-(chunk, block) token lists
    per_core = []
    for c in range(cfg.NC):
        m = (dst // cfg.SHARD) == c
        s, d = src[m], dst[m]
        # split-AG table layout: table half A = all cores' shard rows
        # [0, HROWS), half B = rows [HROWS, SHARD_PAD). AG1 fills rows
        # [0, NC*HROWS) of the A-table, AG2 the B-table.
        score = s // cfg.SHARD
        slocal = s % cfg.SHARD
        shalf = slocal // cfg.HROWS
        srow = (shalf * cfg.NC * cfg.HROWS + score * cfg.HROWS
                + (slocal - shalf * cfg.HROWS))
        chunk = srow // cfg.CHUNK
        lidx = (srow - chunk * cfg.CHUNK).astype(np.int32)
        dl = (d - c * cfg.SHARD).astype(np.int32)
        blk = dl // 128
        dwi = (dl % 128).astype(np.int32)
        order = np.lexsort((dwi, blk, chunk))
        per_core.append((chunk[order], blk[order], lidx[order], dwi[order]))

    # uniform (cross-core max, PADQ-quantized) per-(chunk, block) token counts
    seg_counts = np.zeros((cfg.NC, cfg.NCHUNK, cfg.NBLK), np.int64)
    for c in range(cfg.NC):
        ch, bl, _, _ = per_core[c]
        np.add.at(seg_counts[c], (ch, bl), 1)
    q = cfg.PADQ
    seg_tok = (np.ceil(seg_counts.max(axis=0) / q) * q).astype(np.int64)  # [4,98]
    # token offsets; matmul base partitions must be 0/32/64. Instead of
    # bumping sizes whenever a prefix lands at 96 (mod 128), ORDER the
    # blocks within each chunk so no segment start hits 96 — the sizes are
    # multiples of 32, so greedily pick the next block whose size keeps the
    # running offset off the forbidden residue. Falls back to a +32 bump
    # only when no remaining block fits (rare).
    # The permutation must respect the split-AG phase structure (sessions
    # for chunks 2-3 run dst-blocks < HBLK before >= HBLK), so only permute
    # within those groups; _build re-sorts sessions into column order.
    seg_off = np.zeros((cfg.NCHUNK, cfg.NBLK), np.int64)
    chunk_tok = np.zeros(cfg.NCHUNK, np.int64)
    hb = cfg.NBLK // 2
    off = 0
    for k in range(cfg.NCHUNK):
        k0 = off
        groups = ([list(range(cfg.NBLK))] if k < 2
                  else [list(range(hb)), list(range(hb, cfg.NBLK))])
        for grp in groups:
            remaining = list(grp)
            while remaining:
                pick = None
                for b in remaining:          # prefer natural order
                    if (off + seg_tok[k, b]) % 128 != 96:
                        pick = b
                        break
                if pick is None:
                    pick = remaining[0]
                    seg_tok[k, pick] += 32
                remaining.remove(pick)
                seg_off[k, pick] = off
                off += seg_tok[k, pick]
        off = k0 + -(-(off - k0) // 128) * 128
        chunk_tok[k] = off - k0
    T = int(off)
    ncols = T // 128

    streams = []
    for c in range(cfg.NC):
        ch, bl, lidx, dwi = per_core[c]
        gi = np.zeros(T, np.int16)
        dloc = np.full(T, -1.0, np.float32)
        ei = 0           # edge read position (sorted by (chunk, block))
        for k in range(cfg.NCHUNK):
            for b in range(cfg.NBLK):
                n = int(seg_counts[c, k, b])
                pos = int(seg_off[k, b])
                gi[pos:pos + n] = lidx[ei:ei + n]
                tt = np.arange(pos, pos + n)
                dloc[tt] = dwi[ei:ei + n] + 128.0 * ((tt // 128) % cfg.SELG)
                ei += n
        assert ei == len(lidx)
        gi16 = np.tile(gi.reshape(T // 16, 16).T, (8, 1))            # [128, T/16]
        dloc128 = np.ascontiguousarray(dloc.reshape(T // 128, 128).T)  # [128, T/128]

        def shardvec(v):
            p = np.zeros(cfg.SHARD_PAD, np.float32)
            p[:cfg.SHARD] = v[c * cfg.SHARD:(c + 1) * cfg.SHARD]
            return np.ascontiguousarray(p.reshape(cfg.NBLK, 128).T)   # [128, NBLK]

        streams.append(dict(
            gidx=gi16, dloc=dloc128,
            d2=shardvec(d2), dinv=shardvec(dinv), dinvi=shardvec(dinvi),
        ))
    return streams, (seg_tok, seg_off, chunk_tok, T)


def _plan(cfg, seg_meta):
    """Compile-time plan: gather slices, Sel groups, segment partition-runs."""
    seg_tok, seg_off, chunk_tok, T = seg_meta
    ncols = T // 128
    # gather slices: (chunk, col0, ncol) — within-chunk runs of <= SLICE_COLS
    slices = []
    col = 0
    for k in range(cfg.NCHUNK):
        left = int(chunk_tok[k]) // 128
        while left > 0:
            n = min(cfg.SLICE_COLS, left)
            slices.append((k, col, n))
            col += n
            left -= n
    assert col == ncols
    col2slice = {}
    for si, (k, c0, n) in enumerate(slices):
        for i in range(n):
            col2slice[c0 + i] = (si, i)
    # segments: (chunk, block, [(col, p0, p1, slice_idx, off_in_slice), ...])
    segments = []
    for k in range(cfg.NCHUNK):
        for b in range(cfg.NBLK):
            t0 = int(seg_off[k, b])
            n = int(seg_tok[k, b])
            if n == 0:
                continue
            runs = []
            a = t0
            maxlen = {0: 128, 32: 32, 64: 64}
            while a < t0 + n:
                p = a % 128
                e = min(a + maxlen[p], (a // 128 + 1) * 128, t0 + n)
                col = a // 128
                si, off = col2slice[col]
                runs.append((col, p, p + (e - a), si, off))
                a = e
            segments.append((k, b, runs))
    ngroups = (ncols + cfg.SELG - 1) // cfg.SELG
    return segments, slices, col2slice, ngroups, ncols


# ------------------------------------------------------------- bass graph ----
def _build(cfg, prop_hops, seg_meta):
    from concourse import mybir, bacc, library_config
    import concourse.tile as tile
    from concourse.masks import make_identity

    f32 = mybir.dt.float32
    i16 = mybir.dt.int16
    NBLK, HID, IND, SELG = cfg.NBLK, cfg.HID, cfg.IND, cfg.SELG
    segments, slices, col2slice, ngroups, ncols = _plan(cfg, seg_meta)
    T = ncols * 128
    IOTA_W = SELG * 128

    nc = bacc.Bacc(None, target_bir_lowering=False, debug=False,
                   num_devices=cfg.NC, num_swdge_queues=cfg.NQ,
                   dynamic_dma_scratch_size=int(
                       os.environ.get("GNN_DMASCRATCH", "16384")))

    x_in = nc.declare_dram_parameter("x", [cfg.SHARD_PAD, IND], f32, isOutput=False)
    W0_in = nc.declare_dram_parameter("W0", [IND, HID], f32, isOutput=False)
    Wc_in = nc.declare_dram_parameter("Wcp", [HID, HID], f32, isOutput=False)
    b0_in = nc.declare_dram_parameter("b0b", [128, HID], f32, isOutput=False)
    bc_in = nc.declare_dram_parameter("bcb", [128, HID], f32, isOutput=False)
    d2_in = nc.declare_dram_parameter("d2", [128, NBLK], f32, isOutput=False)
    dv_in = nc.declare_dram_parameter("dinv", [128, NBLK], f32, isOutput=False)
    di_in = nc.declare_dram_parameter("dinvi", [128, NBLK], f32, isOutput=False)
    gi_in = nc.declare_dram_parameter("gidx", [128, T // 16], i16, isOutput=False)
    dl_in = nc.declare_dram_parameter("dloc", [128, T // 128], f32, isOutput=False)
    io_in = nc.declare_dram_parameter("iota", [128, IOTA_W], f32, isOutput=False)
    out_ext = nc.declare_dram_parameter("out", [cfg.SHARD_PAD, HID], f32,
                                        isOutput=True)

    ag_inA = nc.dram_tensor("ag_inA", [cfg.HROWS, HID], f32)
    ag_inB = nc.dram_tensor("ag_inB", [cfg.HROWS, HID], f32)
    tablesA = [
        nc.dram_tensor(f"tableA{i}", [cfg.NC * cfg.HROWS, HID], f32,
                       addr_space="Shared")
        for i in range(cfg.NTABLES)
    ]
    tablesB = [
        nc.dram_tensor(f"tableB{i}", [cfg.NC * cfg.HROWS, HID], f32,
                       addr_space="Shared")
        for i in range(cfg.NTABLES)
    ]
    ag_pmA = ag_inA[:].rearrange("(n p) d -> p n d", p=128)  # [128, HBLK, HID]
    ag_pmB = ag_inB[:].rearrange("(n p) d -> p n d", p=128)
    out_pm = out_ext[:].rearrange("(n p) d -> p n d", p=128)

    with tile.TileContext(nc) as tc:
        with (
            tc.tile_pool(name="const", bufs=1) as cpool,
            tc.tile_pool(name="msg", bufs=8) as mpool,
            tc.tile_pool(name="sel", bufs=4) as lpool,
            tc.tile_pool(name="wk", bufs=2) as wpool,
            tc.tile_pool(name="ps", bufs=2, space="PSUM") as ppool,
            tc.tile_pool(name="pseg", bufs=4, space="PSUM") as gpool,
        ):
            nc.gpsimd.load_library(library_config.mlp)

            gidx = cpool.tile([128, T // 16], i16)
            nc.sync.dma_start(out=gidx[:], in_=gi_in[:])
            dloc = cpool.tile([128, T // 128], f32)
            nc.sync.dma_start(out=dloc[:], in_=dl_in[:])
            iota = cpool.tile([128, IOTA_W], f32)
            nc.sync.dma_start(out=iota[:], in_=io_in[:])
            d2 = cpool.tile([128, NBLK], f32)
            nc.sync.dma_start(out=d2[:], in_=d2_in[:])
            dinv = cpool.tile([128, NBLK], f32)
            nc.sync.dma_start(out=dinv[:], in_=dv_in[:])
            dinvi = cpool.tile([128, NBLK], f32)
            nc.sync.dma_start(out=dinvi[:], in_=di_in[:])
            W0sb = cpool.tile([IND, HID], f32)
            nc.sync.dma_start(out=W0sb[:], in_=W0_in[:])
            Wcsb = cpool.tile([HID, HID], f32)
            nc.sync.dma_start(out=Wcsb[:], in_=Wc_in[:])
            b0sb = cpool.tile([128, HID], f32)
            nc.sync.dma_start(out=b0sb[:], in_=b0_in[:])
            bcsb = cpool.tile([128, HID], f32)
            nc.sync.dma_start(out=bcsb[:], in_=bc_in[:])
            ident = cpool.tile([128, 128], f32)
            make_identity(nc, ident[:])

            acc = cpool.tile([128, NBLK, HID], f32)  # persistent y-shard accum

            # ---- stage A: acc = dinv * (x @ W0) ----
            for j in range(NBLK):
                xt = wpool.tile([128, IND], f32, tag="xt")
                nc.sync.dma_start(out=xt[:], in_=x_in[j * 128:(j + 1) * 128, :])
                ps_t = ppool.tile([128, 128], f32, space="PSUM", tag="pt")
                nc.tensor.transpose(out=ps_t[:], in_=xt[:], identity=ident[:])
                xT = wpool.tile([128, 128], f32, tag="xT")
                nc.vector.tensor_copy(out=xT[:], in_=ps_t[:])
                ps_o = ppool.tile([128, HID], f32, space="PSUM", tag="po")
                nc.tensor.matmul(out=ps_o[:], lhsT=xT[:], rhs=W0sb[:],
                                 start=True, stop=True)
                nc.vector.tensor_scalar(
                    out=acc[:, j, :], in0=ps_o[:], scalar1=dinv[:, j:j + 1],
                    scalar2=None, op0=mybir.AluOpType.mult)

            HBLK = cfg.HBLK

            def issue_agA(h1):
                nc.sync.dma_start(out=ag_pmA, in_=acc[:, :HBLK, :])
                nc.gpsimd.collective_compute(
                    "AllGather", mybir.AluOpType.bypass,
                    replica_groups=[list(range(cfg.NC))],
                    ins=[ag_inA[:]], outs=[tablesA[h1 % cfg.NTABLES][:]])

            def issue_agB(h1):
                nc.sync.dma_start(out=ag_pmB, in_=acc[:, HBLK:, :])
                nc.gpsimd.collective_compute(
                    "AllGather", mybir.AluOpType.bypass,
                    replica_groups=[list(range(cfg.NC))],
                    ins=[ag_inB[:]], outs=[tablesB[h1 % cfg.NTABLES][:]])

            # ---- hops 0..prop_hops (last one = classifier) ----
            for h in range(prop_hops + 1):
                if h == prop_hops:
                    # transform acc: y_z = dinv * (relu(acc*dinvi + b0) @ Wc)
                    nc.vector.tensor_tensor(
                        out=acc[:], in0=acc[:],
                        in1=dinvi[:, :, None].to_broadcast([128, NBLK, HID]),
                        op=mybir.AluOpType.mult)
                    for j in range(NBLK):
                        t = acc[:, j, :]
                        nc.vector.tensor_add(out=t, in0=t, in1=b0sb[:])
                        nc.vector.tensor_relu(out=t, in_=t)
                        ps_t = ppool.tile([128, 128], f32, space="PSUM", tag="pt")
                        nc.tensor.transpose(out=ps_t[:HID, :], in_=t,
                                            identity=ident[:])
                        zT = wpool.tile([HID, 128], f32, tag="xT")
                        nc.vector.tensor_copy(out=zT[:], in_=ps_t[:HID, :])
                        ps_z = ppool.tile([128, HID], f32, space="PSUM", tag="po")
                        nc.tensor.matmul(out=ps_z[:], lhsT=zT[:], rhs=Wcsb[:],
                                         start=True, stop=True)
                        nc.vector.tensor_scalar(
                            out=t, in0=ps_z[:], scalar1=dinv[:, j:j + 1],
                            scalar2=None, op0=mybir.AluOpType.mult)

                if h == 0 or h == prop_hops:
                    # no pipelined AGs available (stage A / transform just ran)
                    issue_agA(h)
                    issue_agB(h)

                tblA = tablesA[h % cfg.NTABLES]
                tblB = tablesB[h % cfg.NTABLES]

                # gathers (per slice), Sel builds (per group), emitted lazily
                msg_tiles = [None] * len(slices)
                sel_tiles = [None] * ngroups

                def get_msg(si):
                    if msg_tiles[si] is None:
                        k, c0, ncol = slices[si]
                        tb = tblA if k < 2 else tblB
                        kk = k % 2
                        mt = mpool.tile([128, cfg.SLICE_COLS, HID], f32,
                                        tag="msg")
                        nc.gpsimd.dma_gather(
                            out_ap=mt[:, :ncol, :],
                            in_ap=tb[kk * cfg.CHUNK:(kk + 1) * cfg.CHUNK, :],
                            idxs_ap=gidx[:, c0 * 8:(c0 + ncol) * 8],
                            num_idxs=ncol * 128,
                            num_idxs_reg=ncol * 128,
                            elem_size=HID,
                            queue_num=si % cfg.NQ,
                            single_packet=not bool(
                                int(os.environ.get("GNN_MULTIPKT", "0"))),
                        )
                        msg_tiles[si] = mt
                    return msg_tiles[si]

                def get_sel(g):
                    if sel_tiles[g] is None:
                        c0 = g * SELG
                        n = min(SELG, ncols - c0)
                        st = lpool.tile([128, IOTA_W], f32, tag="sel")
                        if not cfg.SKIP_SEL:
                            nc.vector.tensor_tensor(
                                out=st[:, :n * 128],
                                in0=dloc[:, c0:c0 + n, None]
                                    .to_broadcast([128, n, 128]),
                                in1=iota[:, :n * 128].rearrange(
                                    "p (n w) -> p n w", w=128),
                                op=mybir.AluOpType.is_equal)
                        sel_tiles[g] = st
                    return sel_tiles[g]

                def do_sessions(pred):
                    segs = [s for s in segments if pred(s[0], s[1])]
                    # physical column order keeps slice/sel access monotone
                    # so the bounded tile pools never recycle a live tile
                    segs.sort(key=lambda s: (s[0], s[2][0][0]))
                    for (k, b, runs) in segs:
                        ps = gpool.tile([128, HID], f32, space="PSUM",
                                        tag="pseg")
                        for i, (col, p0, p1, si, off) in enumerate(runs):
                            mt = get_msg(si)
                            st = get_sel(col // SELG)
                            w = col % SELG
                            nc.tensor.matmul(
                                out=ps[:],
                                lhsT=st[p0:p1, w * 128:(w + 1) * 128],
                                rhs=mt[p0:p1, off, :],
                                start=(i == 0), stop=(i == len(runs) - 1))
                        nc.vector.tensor_add(out=acc[:, b, :],
                                             in0=acc[:, b, :], in1=ps[:])

                # post-scale: y_{h+1} = d2 * acc   (classifier: dinv, + bc)
                scaleA = d2 if h < prop_hops else dinv
                # chunks 0-1 (table half A), all blocks
                do_sessions(lambda k, b: k < 2)
                # chunks 2-3 for dst blocks 0-48 -> acc half A final
                do_sessions(lambda k, b: k >= 2 and b < HBLK)
                nc.vector.tensor_tensor(
                    out=acc[:, :HBLK, :], in0=acc[:, :HBLK, :],
                    in1=scaleA[:, :HBLK, None].to_broadcast([128, HBLK, HID]),
                    op=mybir.AluOpType.mult)
                if h < prop_hops and h + 1 != prop_hops:
                    issue_agA(h + 1)   # overlaps the remaining sessions
                # chunks 2-3 for dst blocks 49-97 -> acc half B final
                do_sessions(lambda k, b: k >= 2 and b >= HBLK)
                nc.vector.tensor_tensor(
                    out=acc[:, HBLK:, :], in0=acc[:, HBLK:, :],
                    in1=scaleA[:, HBLK:, None].to_broadcast(
                        [128, NBLK - HBLK, HID]),
                    op=mybir.AluOpType.mult)
                if h < prop_hops and h + 1 != prop_hops:
                    issue_agB(h + 1)
                if h == prop_hops:
                    nc.vector.tensor_tensor(
                        out=acc[:], in0=acc[:],
                        in1=bcsb[:, None, :].to_broadcast([128, NBLK, HID]),
                        op=mybir.AluOpType.add)

            nc.sync.dma_start(out=out_pm, in_=acc[:])

    nc.compile()
    return nc


# ----------------------------------------------------------------- driver ----
def _make_in_maps(cfg, x, W0, b0, Wc, bc, streams):
    x = np.asarray(x, np.float32)
    W0 = np.asarray(W0, np.float32)
    Wcp = np.zeros((cfg.HID, cfg.HID), np.float32)
    Wcp[:, :cfg.NCLS] = np.asarray(Wc, np.float32)
    b0b = np.tile(np.asarray(b0, np.float32)[None, :], (128, 1))
    bcp = np.zeros(cfg.HID, np.float32)
    bcp[:cfg.NCLS] = np.asarray(bc, np.float32)
    bcb = np.tile(bcp[None, :], (128, 1))
    iota = np.tile(np.arange(cfg.SELG * 128, dtype=np.float32)[None, :],
                   (128, 1))

    in_maps = []
    for c in range(cfg.NC):
        xs = np.zeros((cfg.SHARD_PAD, cfg.IND), np.float32)
        xs[:cfg.SHARD] = x[c * cfg.SHARD:(c + 1) * cfg.SHARD]
        st = streams[c]
        in_maps.append({
            "x": xs, "W0": W0, "Wcp": Wcp, "b0b": b0b, "bcb": bcb,
            "d2": st["d2"], "dinv": st["dinv"], "dinvi": st["dinvi"],
            "gidx": st["gidx"], "dloc": st["dloc"], "iota": iota,
        })
    return in_maps


def _run_timed(nc, in_maps, n_cores, nrep):
    """Mirror of bass2jax.run_bass_via_pjrt's multi-core path, re-executing
    the compiled NEFF `nrep` times to measure warm execute wall time."""
    import time
    import jax
    from concourse import bass2jax, mybir
    bass2jax.install_neuronx_cc_hook()

    partition_name = nc.partition_id_tensor.name if nc.partition_id_tensor else None
    in_names, out_names, out_avals, zero_outs = [], [], [], []
    for alloc in nc.m.functions[0].allocations:
        if not isinstance(alloc, mybir.MemoryLocationSet):
            continue
        name = alloc.memorylocations[0].name
        if alloc.kind == "ExternalInput":
            if name != partition_name:
                in_names.append(name)
        elif alloc.kind == "ExternalOutput":
            out_names.append(name)
            shape = tuple(alloc.tensor_shape)
            dtype = mybir.dt.np(alloc.dtype)
            out_avals.append(jax.core.ShapedArray(shape, dtype))
            zero_outs.append(np.zeros(shape, dtype))
    n_params = len(in_names)
    n_outs = len(out_avals)
    in_names.extend(out_names)
    if partition_name is not None:
        in_names.append(partition_name)

    donate = tuple(range(n_params, n_params + n_outs))

    def _body(*args):
        operands = list(args)
        if partition_name is not None:
            operands.append(bass2jax.partition_id_tensor())
        return tuple(bass2jax._bass_exec_p.bind(
            *operands,
            out_avals=tuple(out_avals),
            in_names=tuple(in_names),
            out_names=tuple(out_names),
            lowering_input_output_aliases=(),
            sim_require_finite=True,
            sim_require_nnan=True,
            nc=nc,
        ))

    devices = jax.devices()[:n_cores]
    mesh = bass2jax.Mesh(np.asarray(devices), ("core",))
    in_specs = (bass2jax.PartitionSpec("core"),) * (n_params + n_outs)
    out_specs = (bass2jax.PartitionSpec("core"),) * len(out_names)
    sharded = jax.jit(
        bass2jax.shard_map(_body, mesh=mesh, in_specs=in_specs,
                           out_specs=out_specs, check_rep=False),
        donate_argnums=donate, keep_unused=True,
    )
    concat_in = [
        np.concatenate([np.asarray(in_maps[c][nm]) for c in range(n_cores)], axis=0)
        for nm in in_names[:n_params]
    ]
    from jax.sharding import NamedSharding
    dev_in = [
        jax.device_put(a, NamedSharding(mesh, bass2jax.PartitionSpec("core")))
        for a in concat_in
    ]
    times = []
    out_arrs = None
    for _ in range(max(1, nrep)):
        dev_zeros = [
            jax.device_put(
                np.zeros((n_cores * z.shape[0], *z.shape[1:]), z.dtype),
                NamedSharding(mesh, bass2jax.PartitionSpec("core")))
            for z in zero_outs
        ]
        for z in dev_zeros:
            z.block_until_ready()
        t0 = time.perf_counter()
        out_arrs = sharded(*dev_in, *dev_zeros)
        for o in out_arrs:
            o.block_until_ready()
        times.append(time.perf_counter() - t0)
    results = [
        {name: np.asarray(out_arrs[i]).reshape(n_cores, *out_avals[i].shape)[c]
         for i, name in enumerate(out_names)}
        for c in range(n_cores)
    ]
    return results, times


def kernel(x, W0, b0, Wc, bc, edge_index, prop_nums, _timeit=0):
    cfg = Cfg()
    prop_hops = int(prop_nums)
    streams, seg_meta = _pack_streams(cfg, np.asarray(edge_index))
    nc = _build(cfg, prop_hops, seg_meta)
    in_maps = _make_in_maps(cfg, x, W0, b0, Wc, bc, streams)

    if _timeit:
        results, times = _run_timed(nc, in_maps, cfg.NC, _timeit)
    else:
        from concourse.bass_utils import run_bass_kernel_spmd
        res = run_bass_kernel_spmd(nc, in_maps, core_ids=list(range(cfg.NC)))
        results = res.results
    out = np.empty((cfg.N, cfg.NCLS), np.float32)
    for c in range(cfg.NC):
        out[c * cfg.SHARD:(c + 1) * cfg.SHARD] = \
            results[c]["out"][:cfg.SHARD, :cfg.NCLS]
    if _timeit:
        return out, times
    return out



# revision 24
# speedup vs baseline: 1.1710x; 1.1710x over previous
"""Distributed Bass kernel for A2GNN propagation (GCN-normalized message
passing) on 8 TRN2 NeuronCores.

Scheme (node/data parallel):
  - Nodes range-sharded across 8 cores (12500 each, padded to 12544 rows).
  - The propagation state is kept in *pre-scaled* form y = dinv * x (the GCN
    norm dinv[s]*dinv[d] is separable), so edge messages need no per-edge
    scaling at all:  x_new[d] = dinv[d] * (sum_{s->d} y[s] + y[d]);
    y_new = dinv^2 * (sum y[s] + y[d]).
  - Per hop: each core AllGathers the 3.2 MB y-shard into a replicated
    [100352, 64] Shared table, dma_gathers the source rows for its edges
    (grouped into 4 table chunks so int16 indices fit), and reduces them
    into its shard accumulator with per-128-dst-block one-hot matmuls on
    the PE (selection matrices built on DVE by is_equal against an iota).
    The accumulator lives in SBUF; its start value (the previous y-shard)
    provides the self-loop term. dma_scatter_add is NOT used (its CCE add
    loses colliding updates on HW).
  - x@W0 and the relu(h+b0)@Wc classifier run on the PE between hop groups.

Perf notes (measured on trn2):
  - The hop is bound by dma_gather descriptor generation: the ANT gather
    ucode runs on ONE Q7 core pair per call (cpu_id/2 == queue_num; tx/rx
    split) at ~6 ns/index + ~1 us fixed, serialized on the Pool engine, so
    each 1024-index call costs ~6 us and 104 calls/hop ~640 us. num_idxs >
    1024 hard-crashes the ucode; multi-packet and indirect_dma_start are
    not faster. SWDGE queues are round-robined (NQ=4) so one queue's DMA
    drain overlaps the next call's descriptor generation (~1.5x).
  - The shard is split into two 49-block halves with separate AllGathers
    (tableA/tableB, double-buffered by hop parity). Each half's AG is
    issued as soon as that half of the accumulator is final, overlapping
    the remaining sessions and the next hop's gathers, which hides the
    whole collective (~84 us/hop exposed otherwise).
"""

import os
import sys

sys.path.insert(0, "/opt/trn_rl_repo")

import numpy as np


# ---------------------------------------------------------------- config ----
class Cfg:
    N = 100000          # nodes
    IND = 128           # input dim
    HID = 64            # hidden dim (and padded classifier dim)
    NCLS = 40           # classes
    NC = 8              # cores
    SHARD = N // NC     # 12500 owned nodes per core
    NBLK = (SHARD + 127) // 128          # 98 row-blocks per shard
    SHARD_PAD = NBLK * 128               # 12544
    TROWS = SHARD_PAD * NC               # 100352 table rows
    HBLK = NBLK // 2                     # 49 blocks per shard half
    HROWS = HBLK * 128                   # 6272 rows per shard half
    NCHUNK = 4
    CHUNK = TROWS // NCHUNK              # 25088 (< 32768 so int16 indices fit)
    SLICE_COLS = int(os.environ.get("GNN_SLICE_COLS", "8"))
    SELG = int(os.environ.get("GNN_SELG", "16"))  # columns per DVE Sel build
    PADQ = 32                            # segment token-count quantum
    NTABLES = int(os.environ.get("GNN_NTABLES", "2"))
    NQ = int(os.environ.get("GNN_NQ", "4"))           # SWDGE queues (1..4)
    SKIP_MM = bool(int(os.environ.get("GNN_SKIP_MM", "0")))
    SKIP_GATHER = bool(int(os.environ.get("GNN_SKIP_GATHER", "0")))
    SKIP_SEL = bool(int(os.environ.get("GNN_SKIP_SEL", "0")))


# ---------------------------------------------------------- host packing ----
def _pack_streams(cfg, edge_index):
    """Per-core token streams: (chunk, dst-block)-segmented, column-padded.

    Returns (streams, seги_meta) where seg_meta = (seg_cols, chunk_cols, T).
    """
    src = np.asarray(edge_index[0], dtype=np.int64)
    dst = np.asarray(edge_index[1], dtype=np.int64)
    N = cfg.N

    deg = (np.bincount(dst, minlength=N) + 1).astype(np.float64)  # + self loop
    dinv64 = 1.0 / np.sqrt(deg)
    dinv = dinv64.astype(np.float32)
    d2 = (dinv64 * dinv64).astype(np.float32)
    dinvi = np.sqrt(deg).astype(np.float32)

    # per-core, per-(chunk, block) token lists
    per_core = []
    for c in range(cfg.NC):
        m = (dst // cfg.SHARD) == c
        s, d = src[m], dst[m]
        # split-AG table layout: table half A = all cores' shard rows
        # [0, HROWS), half B = rows [HROWS, SHARD_PAD). AG1 fills rows
        # [0, NC*HROWS) of the A-table, AG2 the B-table.
        score = s // cfg.SHARD
        slocal = s % cfg.SHARD
        shalf = slocal // cfg.HROWS
        srow = (shalf * cfg.NC * cfg.HROWS + score * cfg.HROWS
                + (slocal - shalf * cfg.HROWS))
        chunk = srow // cfg.CHUNK
        lidx = (srow - chunk * cfg.CHUNK).astype(np.int32)
        dl = (d - c * cfg.SHARD).astype(np.int32)
        blk = dl // 128
        dwi = (dl % 128).astype(np.int32)
        order = np.lexsort((dwi, blk, chunk))
        per_core.append((chunk[order], blk[order], lidx[order], dwi[order]))

    # uniform (cross-core max, PADQ-quantized) per-(chunk, block) token counts
    seg_counts = np.zeros((cfg.NC, cfg.NCHUNK, cfg.NBLK), np.int64)
    for c in range(cfg.NC):
        ch, bl, _, _ = per_core[c]
        np.add.at(seg_counts[c], (ch, bl), 1)
    q = cfg.PADQ
    seg_tok = (np.ceil(seg_counts.max(axis=0) / q) * q).astype(np.int64)  # [4,98]
    # token offsets; matmul base partitions must be 0/32/64. Instead of
    # bumping sizes whenever a prefix lands at 96 (mod 128), ORDER the
    # blocks within each chunk so no segment start hits 96 — the sizes are
    # multiples of 32, so greedily pick the next block whose size keeps the
    # running offset off the forbidden residue. Falls back to a +32 bump
    # only when no remaining block fits (rare).
    # The permutation must respect the split-AG phase structure (sessions
    # for chunks 2-3 run dst-blocks < HBLK before >= HBLK), so only permute
    # within those groups; _build re-sorts sessions into column order.
    seg_off = np.zeros((cfg.NCHUNK, cfg.NBLK), np.int64)
    chunk_tok = np.zeros(cfg.NCHUNK, np.int64)
    hb = cfg.NBLK // 2
    off = 0
    for k in range(cfg.NCHUNK):
        k0 = off
        groups = ([list(range(cfg.NBLK))] if k < 2
                  else [list(range(hb)), list(range(hb, cfg.NBLK))])
        for grp in groups:
            remaining = list(grp)
            while remaining:
                pick = None
                for b in remaining:          # prefer natural order
                    if (off + seg_tok[k, b]) % 128 != 96:
                        pick = b
                        break
                if pick is None:
                    pick = remaining[0]
                    seg_tok[k, pick] += 32
                remaining.remove(pick)
                seg_off[k, pick] = off
                off += seg_tok[k, pick]
        off = k0 + -(-(off - k0) // 128) * 128
        chunk_tok[k] = off - k0
    T = int(off)
    ncols = T // 128

    streams = []
    for c in range(cfg.NC):
        ch, bl, lidx, dwi = per_core[c]
        gi = np.zeros(T, np.int16)
        dloc = np.full(T, -1.0, np.float32)
        ei = 0           # edge read position (sorted by (chunk, block))
        for k in range(cfg.NCHUNK):
            for b in range(cfg.NBLK):
                n = int(seg_counts[c, k, b])
                pos = int(seg_off[k, b])
                gi[pos:pos + n] = lidx[ei:ei + n]
                tt = np.arange(pos, pos + n)
                dloc[tt] = dwi[ei:ei + n] + 128.0 * ((tt // 128) % cfg.SELG)
                ei += n
        assert ei == len(lidx)
        gi16 = np.tile(gi.reshape(T // 16, 16).T, (8, 1))            # [128, T/16]
        dloc128 = np.ascontiguousarray(dloc.reshape(T // 128, 128).T)  # [128, T/128]

        def shardvec(v):
            p = np.zeros(cfg.SHARD_PAD, np.float32)
            p[:cfg.SHARD] = v[c * cfg.SHARD:(c + 1) * cfg.SHARD]
            return np.ascontiguousarray(p.reshape(cfg.NBLK, 128).T)   # [128, NBLK]

        streams.append(dict(
            gidx=gi16, dloc=dloc128,
            d2=shardvec(d2), dinv=shardvec(dinv), dinvi=shardvec(dinvi),
        ))
    return streams, (seg_tok, seg_off, chunk_tok, T)


def _plan(cfg, seg_meta):
    """Compile-time plan: gather slices, Sel groups, segment partition-runs."""
    seg_tok, seg_off, chunk_tok, T = seg_meta
    ncols = T // 128
    # gather slices: (chunk, col0, ncol) — within-chunk runs of <= SLICE_COLS
    slices = []
    col = 0
    for k in range(cfg.NCHUNK):
        left = int(chunk_tok[k]) // 128
        while left > 0:
            n = min(cfg.SLICE_COLS, left)
            slices.append((k, col, n))
            col += n
            left -= n
    assert col == ncols
    col2slice = {}
    for si, (k, c0, n) in enumerate(slices):
        for i in range(n):
            col2slice[c0 + i] = (si, i)
    # segments: (chunk, block, [(col, p0, p1, slice_idx, off_in_slice), ...])
    segments = []
    for k in range(cfg.NCHUNK):
        for b in range(cfg.NBLK):
            t0 = int(seg_off[k, b])
            n = int(seg_tok[k, b])
            if n == 0:
                continue
            runs = []
            a = t0
            maxlen = {0: 128, 32: 32, 64: 64}
            while a < t0 + n:
                p = a % 128
                e = min(a + maxlen[p], (a // 128 + 1) * 128, t0 + n)
                col = a // 128
                si, off = col2slice[col]
                runs.append((col, p, p + (e - a), si, off))
                a = e
            segments.append((k, b, runs))
    ngroups = (ncols + cfg.SELG - 1) // cfg.SELG
    return segments, slices, col2slice, ngroups, ncols


# ------------------------------------------------------------- bass graph ----
def _build(cfg, prop_hops, seg_meta):
    from concourse import mybir, bacc, library_config
    import concourse.tile as tile
    from concourse.masks import make_identity

    f32 = mybir.dt.float32
    i16 = mybir.dt.int16
    NBLK, HID, IND, SELG = cfg.NBLK, cfg.HID, cfg.IND, cfg.SELG
    segments, slices, col2slice, ngroups, ncols = _plan(cfg, seg_meta)
    T = ncols * 128
    IOTA_W = SELG * 128

    nc = bacc.Bacc(None, target_bir_lowering=False, debug=False,
                   num_devices=cfg.NC, num_swdge_queues=cfg.NQ,
                   dynamic_dma_scratch_size=int(
                       os.environ.get("GNN_DMASCRATCH", "16384")))

    x_in = nc.declare_dram_parameter("x", [cfg.SHARD_PAD, IND], f32, isOutput=False)
    W0_in = nc.declare_dram_parameter("W0", [IND, HID], f32, isOutput=False)
    Wc_in = nc.declare_dram_parameter("Wcp", [HID, HID], f32, isOutput=False)
    b0_in = nc.declare_dram_parameter("b0b", [128, HID], f32, isOutput=False)
    bc_in = nc.declare_dram_parameter("bcb", [128, HID], f32, isOutput=False)
    d2_in = nc.declare_dram_parameter("d2", [128, NBLK], f32, isOutput=False)
    dv_in = nc.declare_dram_parameter("dinv", [128, NBLK], f32, isOutput=False)
    di_in = nc.declare_dram_parameter("dinvi", [128, NBLK], f32, isOutput=False)
    gi_in = nc.declare_dram_parameter("gidx", [128, T // 16], i16, isOutput=False)
    dl_in = nc.declare_dram_parameter("dloc", [128, T // 128], f32, isOutput=False)
    io_in = nc.declare_dram_parameter("iota", [128, IOTA_W], f32, isOutput=False)
    out_ext = nc.declare_dram_parameter("out", [cfg.SHARD_PAD, HID], f32,
                                        isOutput=True)

    ag_inA = nc.dram_tensor("ag_inA", [cfg.HROWS, HID], f32)
    ag_inB = nc.dram_tensor("ag_inB", [cfg.HROWS, HID], f32)
    tablesA = [
        nc.dram_tensor(f"tableA{i}", [cfg.NC * cfg.HROWS, HID], f32,
                       addr_space="Shared")
        for i in range(cfg.NTABLES)
    ]
    tablesB = [
        nc.dram_tensor(f"tableB{i}", [cfg.NC * cfg.HROWS, HID], f32,
                       addr_space="Shared")
        for i in range(cfg.NTABLES)
    ]
    ag_pmA = ag_inA[:].rearrange("(n p) d -> p n d", p=128)  # [128, HBLK, HID]
    ag_pmB = ag_inB[:].rearrange("(n p) d -> p n d", p=128)
    out_pm = out_ext[:].rearrange("(n p) d -> p n d", p=128)

    with tile.TileContext(nc) as tc:
        with (
            tc.tile_pool(name="const", bufs=1) as cpool,
            tc.tile_pool(name="msg", bufs=8) as mpool,
            tc.tile_pool(name="sel", bufs=4) as lpool,
            tc.tile_pool(name="wk", bufs=2) as wpool,
            tc.tile_pool(name="ps", bufs=2, space="PSUM") as ppool,
            tc.tile_pool(name="pseg", bufs=4, space="PSUM") as gpool,
        ):
            nc.gpsimd.load_library(library_config.mlp)

            gidx = cpool.tile([128, T // 16], i16)
            nc.sync.dma_start(out=gidx[:], in_=gi_in[:])
            dloc = cpool.tile([128, T // 128], f32)
            nc.sync.dma_start(out=dloc[:], in_=dl_in[:])
            iota = cpool.tile([128, IOTA_W], f32)
            nc.sync.dma_start(out=iota[:], in_=io_in[:])
            d2 = cpool.tile([128, NBLK], f32)
            nc.sync.dma_start(out=d2[:], in_=d2_in[:])
            dinv = cpool.tile([128, NBLK], f32)
            nc.sync.dma_start(out=dinv[:], in_=dv_in[:])
            dinvi = cpool.tile([128, NBLK], f32)
            nc.sync.dma_start(out=dinvi[:], in_=di_in[:])
            W0sb = cpool.tile([IND, HID], f32)
            nc.sync.dma_start(out=W0sb[:], in_=W0_in[:])
            Wcsb = cpool.tile([HID, HID], f32)
            nc.sync.dma_start(out=Wcsb[:], in_=Wc_in[:])
            b0sb = cpool.tile([128, HID], f32)
            nc.sync.dma_start(out=b0sb[:], in_=b0_in[:])
            bcsb = cpool.tile([128, HID], f32)
            nc.sync.dma_start(out=bcsb[:], in_=bc_in[:])
            ident = cpool.tile([128, 128], f32)
            make_identity(nc, ident[:])

            acc = cpool.tile([128, NBLK, HID], f32)  # persistent y-shard accum

            # ---- stage A: acc = dinv * (x @ W0) ----
            for j in range(NBLK):
                xt = wpool.tile([128, IND], f32, tag="xt")
                nc.sync.dma_start(out=xt[:], in_=x_in[j * 128:(j + 1) * 128, :])
                ps_t = ppool.tile([128, 128], f32, space="PSUM", tag="pt")
                nc.tensor.transpose(out=ps_t[:], in_=xt[:], identity=ident[:])
                xT = wpool.tile([128, 128], f32, tag="xT")
                nc.vector.tensor_copy(out=xT[:], in_=ps_t[:])
                ps_o = ppool.tile([128, HID], f32, space="PSUM", tag="po")
                nc.tensor.matmul(out=ps_o[:], lhsT=xT[:], rhs=W0sb[:],
                                 start=True, stop=True)
                nc.vector.tensor_scalar(
                    out=acc[:, j, :], in0=ps_o[:], scalar1=dinv[:, j:j + 1],
                    scalar2=None, op0=mybir.AluOpType.mult)

            HBLK = cfg.HBLK

            # hoist the num_idxs constants: one RegisterMove each instead of
            # one per gather call (115/hop on the serial Pool sequencer)
            nidx_regs = {}
            for (_k, _c0, _ncol) in slices:
                v = _ncol * 128
                if v not in nidx_regs:
                    nidx_regs[v] = nc.gpsimd.to_reg(v)

            def issue_agA(h1):
                nc.sync.dma_start(out=ag_pmA, in_=acc[:, :HBLK, :])
                nc.gpsimd.collective_compute(
                    "AllGather", mybir.AluOpType.bypass,
                    replica_groups=[list(range(cfg.NC))],
                    ins=[ag_inA[:]], outs=[tablesA[h1 % cfg.NTABLES][:]])

            def issue_agB(h1):
                nc.sync.dma_start(out=ag_pmB, in_=acc[:, HBLK:, :])
                nc.gpsimd.collective_compute(
                    "AllGather", mybir.AluOpType.bypass,
                    replica_groups=[list(range(cfg.NC))],
                    ins=[ag_inB[:]], outs=[tablesB[h1 % cfg.NTABLES][:]])

            # ---- hops 0..prop_hops (last one = classifier) ----
            for h in range(prop_hops + 1):
                if h == prop_hops:
                    # transform acc: y_z = dinv * (relu(acc*dinvi + b0) @ Wc)
                    nc.vector.tensor_tensor(
                        out=acc[:], in0=acc[:],
                        in1=dinvi[:, :, None].to_broadcast([128, NBLK, HID]),
                        op=mybir.AluOpType.mult)
                    for j in range(NBLK):
                        t = acc[:, j, :]
                        nc.vector.tensor_add(out=t, in0=t, in1=b0sb[:])
                        nc.vector.tensor_relu(out=t, in_=t)
                        ps_t = ppool.tile([128, 128], f32, space="PSUM", tag="pt")
                        nc.tensor.transpose(out=ps_t[:HID, :], in_=t,
                                            identity=ident[:])
                        zT = wpool.tile([HID, 128], f32, tag="xT")
                        nc.vector.tensor_copy(out=zT[:], in_=ps_t[:HID, :])
                        ps_z = ppool.tile([128, HID], f32, space="PSUM", tag="po")
                        nc.tensor.matmul(out=ps_z[:], lhsT=zT[:], rhs=Wcsb[:],
                                         start=True, stop=True)
                        nc.vector.tensor_scalar(
                            out=t, in0=ps_z[:], scalar1=dinv[:, j:j + 1],
                            scalar2=None, op0=mybir.AluOpType.mult)

                if h == 0 or h == prop_hops:
                    # no pipelined AGs available (stage A / transform just ran)
                    issue_agA(h)
                    issue_agB(h)

                tblA = tablesA[h % cfg.NTABLES]
                tblB = tablesB[h % cfg.NTABLES]

                # gathers (per slice), Sel builds (per group), emitted lazily
                msg_tiles = [None] * len(slices)
                sel_tiles = [None] * ngroups

                def get_msg(si):
                    if msg_tiles[si] is None:
                        k, c0, ncol = slices[si]
                        tb = tblA if k < 2 else tblB
                        kk = k % 2
                        mt = mpool.tile([128, cfg.SLICE_COLS, HID], f32,
                                        tag="msg")
                        nc.gpsimd.dma_gather(
                            out_ap=mt[:, :ncol, :],
                            in_ap=tb[kk * cfg.CHUNK:(kk + 1) * cfg.CHUNK, :],
                            idxs_ap=gidx[:, c0 * 8:(c0 + ncol) * 8],
                            num_idxs=ncol * 128,
                            num_idxs_reg=nidx_regs[ncol * 128],
                            elem_size=HID,
                            queue_num=si % cfg.NQ,
                            single_packet=not bool(
                                int(os.environ.get("GNN_MULTIPKT", "0"))),
                        )
                        msg_tiles[si] = mt
                    return msg_tiles[si]

                def get_sel(g):
                    if sel_tiles[g] is None:
                        c0 = g * SELG
                        n = min(SELG, ncols - c0)
                        st = lpool.tile([128, IOTA_W], f32, tag="sel")
                        if not cfg.SKIP_SEL:
                            nc.vector.tensor_tensor(
                                out=st[:, :n * 128],
                                in0=dloc[:, c0:c0 + n, None]
                                    .to_broadcast([128, n, 128]),
                                in1=iota[:, :n * 128].rearrange(
                                    "p (n w) -> p n w", w=128),
                                op=mybir.AluOpType.is_equal)
                        sel_tiles[g] = st
                    return sel_tiles[g]

                def do_sessions(pred):
                    segs = [s for s in segments if pred(s[0], s[1])]
                    # physical column order keeps slice/sel access monotone
                    # so the bounded tile pools never recycle a live tile
                    segs.sort(key=lambda s: (s[0], s[2][0][0]))
                    for (k, b, runs) in segs:
                        ps = gpool.tile([128, HID], f32, space="PSUM",
                                        tag="pseg")
                        for i, (col, p0, p1, si, off) in enumerate(runs):
                            mt = get_msg(si)
                            st = get_sel(col // SELG)
                            w = col % SELG
                            nc.tensor.matmul(
                                out=ps[:],
                                lhsT=st[p0:p1, w * 128:(w + 1) * 128],
                                rhs=mt[p0:p1, off, :],
                                start=(i == 0), stop=(i == len(runs) - 1))
                        nc.vector.tensor_add(out=acc[:, b, :],
                                             in0=acc[:, b, :], in1=ps[:])

                # post-scale: y_{h+1} = d2 * acc   (classifier: dinv, + bc)
                scaleA = d2 if h < prop_hops else dinv
                # chunks 0-1 (table half A), all blocks
                do_sessions(lambda k, b: k < 2)
                # chunks 2-3 for dst blocks 0-48 -> acc half A final
                do_sessions(lambda k, b: k >= 2 and b < HBLK)
                nc.vector.tensor_tensor(
                    out=acc[:, :HBLK, :], in0=acc[:, :HBLK, :],
                    in1=scaleA[:, :HBLK, None].to_broadcast([128, HBLK, HID]),
                    op=mybir.AluOpType.mult)
                if h < prop_hops and h + 1 != prop_hops:
                    issue_agA(h + 1)   # overlaps the remaining sessions
                # chunks 2-3 for dst blocks 49-97 -> acc half B final
                do_sessions(lambda k, b: k >= 2 and b >= HBLK)
                nc.vector.tensor_tensor(
                    out=acc[:, HBLK:, :], in0=acc[:, HBLK:, :],
                    in1=scaleA[:, HBLK:, None].to_broadcast(
                        [128, NBLK - HBLK, HID]),
                    op=mybir.AluOpType.mult)
                if h < prop_hops and h + 1 != prop_hops:
                    issue_agB(h + 1)
                if h == prop_hops:
                    nc.vector.tensor_tensor(
                        out=acc[:], in0=acc[:],
                        in1=bcsb[:, None, :].to_broadcast([128, NBLK, HID]),
                        op=mybir.AluOpType.add)

            nc.sync.dma_start(out=out_pm, in_=acc[:])

    nc.compile()
    return nc


# ----------------------------------------------------------------- driver ----
def _make_in_maps(cfg, x, W0, b0, Wc, bc, streams):
    x = np.asarray(x, np.float32)
    W0 = np.asarray(W0, np.float32)
    Wcp = np.zeros((cfg.HID, cfg.HID), np.float32)
    Wcp[:, :cfg.NCLS] = np.asarray(Wc, np.float32)
    b0b = np.tile(np.asarray(b0, np.float32)[None, :], (128, 1))
    bcp = np.zeros(cfg.HID, np.float32)
    bcp[:cfg.NCLS] = np.asarray(bc, np.float32)
    bcb = np.tile(bcp[None, :], (128, 1))
    iota = np.tile(np.arange(cfg.SELG * 128, dtype=np.float32)[None, :],
                   (128, 1))

    in_maps = []
    for c in range(cfg.NC):
        xs = np.zeros((cfg.SHARD_PAD, cfg.IND), np.float32)
        xs[:cfg.SHARD] = x[c * cfg.SHARD:(c + 1) * cfg.SHARD]
        st = streams[c]
        in_maps.append({
            "x": xs, "W0": W0, "Wcp": Wcp, "b0b": b0b, "bcb": bcb,
            "d2": st["d2"], "dinv": st["dinv"], "dinvi": st["dinvi"],
            "gidx": st["gidx"], "dloc": st["dloc"], "iota": iota,
        })
    return in_maps


def _run_timed(nc, in_maps, n_cores, nrep):
    """Mirror of bass2jax.run_bass_via_pjrt's multi-core path, re-executing
    the compiled NEFF `nrep` times to measure warm execute wall time."""
    import time
    import jax
    from concourse import bass2jax, mybir
    bass2jax.install_neuronx_cc_hook()

    partition_name = nc.partition_id_tensor.name if nc.partition_id_tensor else None
    in_names, out_names, out_avals, zero_outs = [], [], [], []
    for alloc in nc.m.functions[0].allocations:
        if not isinstance(alloc, mybir.MemoryLocationSet):
            continue
        name = alloc.memorylocations[0].name
        if alloc.kind == "ExternalInput":
            if name != partition_name:
                in_names.append(name)
        elif alloc.kind == "ExternalOutput":
            out_names.append(name)
            shape = tuple(alloc.tensor_shape)
            dtype = mybir.dt.np(alloc.dtype)
            out_avals.append(jax.core.ShapedArray(shape, dtype))
            zero_outs.append(np.zeros(shape, dtype))
    n_params = len(in_names)
    n_outs = len(out_avals)
    in_names.extend(out_names)
    if partition_name is not None:
        in_names.append(partition_name)

    donate = tuple(range(n_params, n_params + n_outs))

    def _body(*args):
        operands = list(args)
        if partition_name is not None:
            operands.append(bass2jax.partition_id_tensor())
        return tuple(bass2jax._bass_exec_p.bind(
            *operands,
            out_avals=tuple(out_avals),
            in_names=tuple(in_names),
            out_names=tuple(out_names),
            lowering_input_output_aliases=(),
            sim_require_finite=True,
            sim_require_nnan=True,
            nc=nc,
        ))

    devices = jax.devices()[:n_cores]
    mesh = bass2jax.Mesh(np.asarray(devices), ("core",))
    in_specs = (bass2jax.PartitionSpec("core"),) * (n_params + n_outs)
    out_specs = (bass2jax.PartitionSpec("core"),) * len(out_names)
    sharded = jax.jit(
        bass2jax.shard_map(_body, mesh=mesh, in_specs=in_specs,
                           out_specs=out_specs, check_rep=False),
        donate_argnums=donate, keep_unused=True,
    )
    concat_in = [
        np.concatenate([np.asarray(in_maps[c][nm]) for c in range(n_cores)], axis=0)
        for nm in in_names[:n_params]
    ]
    from jax.sharding import NamedSharding
    dev_in = [
        jax.device_put(a, NamedSharding(mesh, bass2jax.PartitionSpec("core")))
        for a in concat_in
    ]
    times = []
    out_arrs = None
    for _ in range(max(1, nrep)):
        dev_zeros = [
            jax.device_put(
                np.zeros((n_cores * z.shape[0], *z.shape[1:]), z.dtype),
                NamedSharding(mesh, bass2jax.PartitionSpec("core")))
            for z in zero_outs
        ]
        for z in dev_zeros:
            z.block_until_ready()
        t0 = time.perf_counter()
        out_arrs = sharded(*dev_in, *dev_zeros)
        for o in out_arrs:
            o.block_until_ready()
        times.append(time.perf_counter() - t0)
    results = [
        {name: np.asarray(out_arrs[i]).reshape(n_cores, *out_avals[i].shape)[c]
         for i, name in enumerate(out_names)}
        for c in range(n_cores)
    ]
    return results, times


def kernel(x, W0, b0, Wc, bc, edge_index, prop_nums, _timeit=0):
    cfg = Cfg()
    prop_hops = int(prop_nums)
    streams, seg_meta = _pack_streams(cfg, np.asarray(edge_index))
    nc = _build(cfg, prop_hops, seg_meta)
    in_maps = _make_in_maps(cfg, x, W0, b0, Wc, bc, streams)

    if _timeit:
        results, times = _run_timed(nc, in_maps, cfg.NC, _timeit)
    else:
        from concourse.bass_utils import run_bass_kernel_spmd
        res = run_bass_kernel_spmd(nc, in_maps, core_ids=list(range(cfg.NC)))
        results = res.results
    out = np.empty((cfg.N, cfg.NCLS), np.float32)
    for c in range(cfg.NC):
        out[c * cfg.SHARD:(c + 1) * cfg.SHARD] = \
            results[c]["out"][:cfg.SHARD, :cfg.NCLS]
    if _timeit:
        return out, times
    return out

